# revision 37
# baseline (speedup 1.0000x reference)
"""Multi-head causal attention (B=2, S=2048, D=1024, H=16) on 8 trn2 cores.

Sharding: core c -> batch b=c//4, head-group g=c%4 (heads 4g..4g+3).

Host I/O is minimized: each core uploads only a distinct 1 MiB chunk of
xT[b] (AllGather over the 4-core batch group rebuilds the full xT on
device) and half of its head-group weight blob (pair AllGather over
{g, g+4} rebuilds the full blob). The out-projection partials are written
as fp16 and ReduceScatter(add)-ed over the batch group, so each core
downloads only its own 512-row slice of the final output.

Each core: Q/K/V projections for its heads from xT[b], causal attention in
transposed layout, row-parallel out-projection partial (bias is fed as
zeros to 3 of the 4 cores and applied on-device via a K=1 broadcast
matmul, so it survives the cross-core reduction exactly once).
"""

import numpy as np

import concourse.bass as bass
import concourse.tile as tile
import concourse.mybir as mybir
from concourse import bacc
from concourse.bass_utils import run_bass_kernel_spmd

B, S, D, H, DH = 2, 2048, 1024, 16, 64
NCORES = 8
HPC = 4          # heads per core
PAIRS = 2        # head pairs per core
QT = 512         # q tile (free dim of scoresT / PV matmuls)
KB = 128         # k block (partition dim of scoresT)
NQT = S // QT    # 4
NKB = S // KB    # 16
DC = D // 128    # 8 contraction chunks for projections
SCALE = 1.0 / np.sqrt(DH)

F32 = mybir.dt.float32
F16 = mybir.dt.float16
BF = mybir.dt.bfloat16

G4 = [[0, 1, 2, 3], [4, 5, 6, 7]]      # batch groups (reduce heads)
G2 = [[0, 4], [1, 5], [2, 6], [3, 7]]  # weight pairs (same head-group)

# bench-only ablation switches (set of strings)
ABLATE = set()


def _build():
    nc = bacc.Bacc("TRN2", target_bir_lowering=False, debug=False, num_devices=NCORES)

    # per-core distinct uploads (1 MiB each)
    x_u = nc.dram_tensor("x_u", [256, S], BF, kind="ExternalInput").ap()
    w_u = nc.dram_tensor("w_u", [2048, 256], BF, kind="ExternalInput").ap()
    bo_r = nc.dram_tensor("bo_r", [1, D], BF, kind="ExternalInput").ap()
    # per-core distinct download: rows 512g..512(g+1) of batch b, fp16
    out = nc.dram_tensor("out", [S // 4, D], F16, kind="ExternalOutput").ap()

    with tile.TileContext(nc) as tc, \
         tc.tile_pool(name="dram", bufs=1, space="DRAM") as dram, \
         tc.tile_pool(name="persist", bufs=1) as persist:
        # ---- internal DRAM (collective operands must not be I/O tensors) ----
        # x AllGather is split into S-halves so projections can start on the
        # left half while the right half is still on the wire.
        x_bl = dram.tile([256, S // 2], BF, name="x_bl")
        x_br = dram.tile([256, S // 2], BF, name="x_br")
        xT_l = dram.tile([D, S // 2], BF, name="xT_l")
        xT_r = dram.tile([D, S // 2], BF, name="xT_r")
        # weight pair exchange: b=0 cores upload [Wq; Wk] slices, b=1 cores
        # upload [Wv; Wo-flat]; one pair AllGather rebuilds the full blob
        w_b = dram.tile([2048, 256], BF, name="w_b")
        wfull = dram.tile([4096, 256], BF, name="wfull")
        # out-proj partials + ReduceScatter results, one pair per half q tile
        # (2 qb blocks) so each RS only depends on its own writes and the
        # final RS tail is half-sized
        NRS = 2 * NQT
        parts = [dram.tile([QT // 2, D], F16, name=f"part{t}") for t in range(NRS)]
        rss = [dram.tile([KB // 2, D], F16, name=f"rs{t}") for t in range(NRS)]
        warm_in = dram.tile([1, 512], BF, name="warm_in")
        warm4 = dram.tile([4, 512], BF, name="warm4")
        warm2 = dram.tile([2, 512], BF, name="warm2")

        # tiny dummy collectives fire first: they pay the ncfw entry latency
        # while the bounce DMAs below are still running
        nc.gpsimd.dma_start(warm_in[:], bo_r[:, 0:512])
        nc.gpsimd.collective_compute(
            "AllGather", mybir.AluOpType.bypass, replica_groups=G2,
            ins=[warm_in.opt()], outs=[warm2.opt()])
        nc.gpsimd.collective_compute(
            "AllGather", mybir.AluOpType.bypass, replica_groups=G4,
            ins=[warm_in.opt()], outs=[warm4.opt()])

        # bounces go on the scalar HWDGE ring so queued input loads (which
        # wait on the AllGather semaphores) can't head-of-line block them
        nc.scalar.dma_start(w_b[:], w_u[:])
        nc.scalar.dma_start(x_bl[:], x_u[:, 0:S // 2])
        nc.scalar.dma_start(x_br[:], x_u[:, S // 2:S])
        nc.gpsimd.collective_compute(
            "AllGather", mybir.AluOpType.bypass, replica_groups=G2,
            ins=[w_b.opt()], outs=[wfull.opt()])
        nc.gpsimd.collective_compute(
            "AllGather", mybir.AluOpType.bypass, replica_groups=G4,
            ins=[x_bl.opt()], outs=[xT_l.opt()])
        nc.gpsimd.collective_compute(
            "AllGather", mybir.AluOpType.bypass, replica_groups=G4,
            ins=[x_br.opt()], outs=[xT_r.opt()])

        # ---- persistent tiles ----
        qt_sb = [persist.tile([128, S], BF, name=f"qt{p}", tag=f"qt{p}") for p in range(PAIRS)]
        kt_sb = [persist.tile([128, S], BF, name=f"kt{p}", tag=f"kt{p}") for p in range(PAIRS)]
        # V' tiles: per s-block j, [128, 4*65]; head hl at cols 65*hl, ones col at 65*hl+64
        vt_sb = [persist.tile([128, HPC * (DH + 1)], BF, name=f"vt{j}", tag=f"vt{j}") for j in range(NKB)]
        ctx_sb = [persist.tile([128, S], BF, name=f"ctx{p}", tag=f"ctx{p}") for p in range(PAIRS)]
        wo_sb = [persist.tile([128, D], BF, name=f"wo{p}", tag=f"wo{p}") for p in range(PAIRS)]
        tri_sb = persist.tile([KB, KB], BF, name="tri", tag="tri")
        bo_sb = persist.tile([1, D], BF, name="bo", tag="bo")
        ones_sb = persist.tile([1, 128], BF, name="ones", tag="ones")

        xts = [persist.tile([128, S], BF, name=f"xts{i}", tag=f"xts{i}") for i in range(DC)]
        wq_sb = [persist.tile([128, HPC * DH], BF, name=f"wq{i}", tag=f"wq{i}") for i in range(DC)]
        wk_sb = [persist.tile([128, HPC * DH], BF, name=f"wk{i}", tag=f"wk{i}") for i in range(DC)]
        wv_sb = [persist.tile([128, HPC * DH], BF, name=f"wv{i}", tag=f"wv{i}") for i in range(DC)]

        # lower-triangular [128,128] mask built on device: keep where q-k >= 0
        tri_ones = persist.tile([KB, KB], BF, name="tri1", tag="tri1")
        nc.gpsimd.memset(tri_ones[:], 1.0)
        nc.gpsimd.affine_select(
            tri_sb[:], tri_ones[:], pattern=[[1, KB]],
            compare_op=mybir.AluOpType.is_ge, fill=0.0, base=0,
            channel_multiplier=-1)
        nc.sync.dma_start(bo_sb[:], bo_r[:])
        nc.gpsimd.memset(ones_sb[:], 1.0)
        for i in range(DC):
            nc.sync.dma_start(xts[i][:, 0:S // 2], xT_l[i * 128:(i + 1) * 128, :])
            nc.sync.dma_start(wq_sb[i][:], wfull[i * 128:(i + 1) * 128, :])
            nc.sync.dma_start(wk_sb[i][:], wfull[1024 + i * 128:1024 + (i + 1) * 128, :])
            nc.sync.dma_start(wv_sb[i][:], wfull[2048 + i * 128:2048 + (i + 1) * 128, :])
        for i in range(DC):
            nc.sync.dma_start(xts[i][:, S // 2:S], xT_r[i * 128:(i + 1) * 128, :])
        for p in range(PAIRS):
            # Wo slice rows 128p..128(p+1) of [256, D], stored flat as
            # [512, 256] rows 3072+512p.. of the blob
            nc.sync.dma_start(
                wo_sb[p].rearrange("q (u f) -> q u f", u=4),
                wfull[3072 + 512 * p:3072 + 512 * (p + 1), :].rearrange(
                    "(q u) f -> q u f", u=4))

        def proj_qk_pass(p, pool, sts):
            """q/k projection for pair p over q tiles `sts`, D-chunk-outer so
            matmuls chase the xT DMAs chunk by chunk. Tags are shared between
            passes so both passes fit in 4 psum banks (WAR deps via Tile)."""
            qps = {st: pool.tile([128, QT], F32, name=f"qps{st}", tag=f"qk{st % 2}")
                   for st in sts}
            kps = {st: pool.tile([128, QT], F32, name=f"kps{st}", tag=f"qk{2 + st % 2}")
                   for st in sts}
            for i in range(DC):
                for st in sts:
                    nc.tensor.matmul(
                        qps[st][:], wq_sb[i][:, p * 128:(p + 1) * 128],
                        xts[i][:, st * QT:(st + 1) * QT],
                        start=(i == 0), stop=(i == DC - 1))
                for st in sts:
                    nc.tensor.matmul(
                        kps[st][:], wk_sb[i][:, p * 128:(p + 1) * 128],
                        xts[i][:, st * QT:(st + 1) * QT],
                        start=(i == 0), stop=(i == DC - 1))
            for st in sts:
                nc.scalar.copy(qt_sb[p][:, st * QT:(st + 1) * QT], qps[st][:])
                nc.vector.tensor_copy(kt_sb[p][:, st * QT:(st + 1) * QT], kps[st][:])

        def proj_qk_seq(p, pool):
            """q/k projection, sequential psum (2 banks) — for overlap with
            attention of the other pair."""
            for st in range(NQT):
                qp = pool.tile([128, QT], F32, name="qp", tag="qkseq")
                for i in range(DC):
                    nc.tensor.matmul(
                        qp[:], wq_sb[i][:, p * 128:(p + 1) * 128],
                        xts[i][:, st * QT:(st + 1) * QT],
                        start=(i == 0), stop=(i == DC - 1))
                nc.scalar.copy(qt_sb[p][:, st * QT:(st + 1) * QT], qp[:])
                kp = pool.tile([128, QT], F32, name="kp", tag="qkseq")
                for i in range(DC):
                    nc.tensor.matmul(
                        kp[:], wk_sb[i][:, p * 128:(p + 1) * 128],
                        xts[i][:, st * QT:(st + 1) * QT],
                        start=(i == 0), stop=(i == DC - 1))
                nc.vector.tensor_copy(kt_sb[p][:, st * QT:(st + 1) * QT], kp[:])

        def attention(p, h, qt_i, scps, ctxps, att, attsm):
            hl = 2 * p + h
            r0, r1 = h * 64, h * 64 + 64
            q0 = qt_i * QT
            nkb = 4 * (qt_i + 1)
            cps = ctxps.tile([DH + 1, QT], F32, name="cps", tag="cps")
            for g0 in range(0, nkb, 2):
                sp = scps.tile([128, 2 * QT], F32, name="sp", tag="sp")
                for u in range(2):
                    kb = g0 + u
                    nc.tensor.matmul(
                        sp[:, u * QT:(u + 1) * QT],
                        kt_sb[p][r0:r1, kb * KB:(kb + 1) * KB],
                        qt_sb[p][r0:r1, q0:q0 + QT],
                        start=True, stop=True)
                pt = att.tile([128, 2 * QT], BF, name="pt", tag="pt")
                nc.scalar.activation(
                    pt[:], sp[:], mybir.ActivationFunctionType.Exp,
                    scale=float(SCALE))
                # causal masking: straddle groups are exactly g0==4qt (d=0,1)
                # and g0==4qt+2 (d=2,3): memset the dead rectangles (gpsimd),
                # multiply the [128,128] diagonal triangles (DVE)
                if "masks" in ABLATE:
                    pass
                elif g0 == 4 * qt_i:          # blocks d=0, d=1
                    nc.gpsimd.memset(pt[:, QT:QT + KB], 0.0)
                    for off in (0, QT + KB):
                        nc.vector.tensor_mul(
                            pt[:, off:off + KB], pt[:, off:off + KB], tri_sb[:])
                elif g0 == 4 * qt_i + 2:    # blocks d=2, d=3
                    nc.gpsimd.memset(pt[:, 0:2 * KB], 0.0)
                    nc.gpsimd.memset(pt[:, QT:QT + 3 * KB], 0.0)
                    for off in (2 * KB, QT + 3 * KB):
                        nc.vector.tensor_mul(
                            pt[:, off:off + KB], pt[:, off:off + KB], tri_sb[:])
                for u in range(2):
                    kb = g0 + u
                    nc.tensor.matmul(
                        cps[:],
                        vt_sb[kb][:, hl * (DH + 1):(hl + 1) * (DH + 1)],
                        pt[:, u * QT:(u + 1) * QT],
                        start=(kb == 0), stop=(kb == nkb - 1))
            # normalize: r = 1/l broadcast over the 64 ctx rows
            r_sb = attsm.tile([1, QT], F32, name="r_sb", tag="r")
            nc.vector.reciprocal(r_sb[:], cps[DH:DH + 1, :])
            rb = attsm.tile([64, QT], F32, name="rb", tag="rb")
            nc.gpsimd.partition_broadcast(rb[:], r_sb[:])
            nc.vector.tensor_mul(
                ctx_sb[p][r0:r1, q0:q0 + QT], cps[0:DH, :], rb[:])

        def outproj(qt_i, ph3ps, ph3sb):
            """partial out-projection rows for one q tile, bias folded in as a
            K=1 matmul; copy PSUM->SBUF (fp16 cast) split across ACT/DVE, then
            DMA to the partials buffer; a half-tile ReduceScatter is issued
            after every 2 qb blocks so the final RS tail is half-sized."""
            for qb in range(qt_i * 4, qt_i * 4 + 4):
                os_ = ph3sb.tile([128, D], F16, name="os", tag="os")
                for nh in range(2):
                    op = ph3ps.tile([128, 512], F32, name="op", tag="op")
                    nc.tensor.matmul(
                        op[:], ones_sb[:], bo_sb[:, nh * 512:(nh + 1) * 512],
                        start=True, stop=False)
                    for p in range(PAIRS):
                        nc.tensor.matmul(
                            op[:], ctx_sb[p][:, qb * 128:(qb + 1) * 128],
                            wo_sb[p][:, nh * 512:(nh + 1) * 512],
                            start=False, stop=(p == PAIRS - 1))
                    dst = os_[:, nh * 512:(nh + 1) * 512]
                    if nh == 0:
                        nc.scalar.copy(dst, op[:])
                    else:
                        nc.vector.tensor_copy(dst, op[:])
                qh = qb // 2
                nc.sync.dma_start(parts[qh][(qb % 2) * 128:(qb % 2) * 128 + 128, :], os_[:])
                if qb % 2 == 1:
                    rs_half(qh)

        def rs_half(qh):
            """reduce one half q tile's partials across the batch group; rank
            g keeps rows 64g..64(g+1), written to output rows 64*qh.."""
            nc.gpsimd.collective_compute(
                "ReduceScatter", mybir.AluOpType.add, replica_groups=G4,
                ins=[parts[qh].opt()], outs=[rss[qh].opt()])
            nc.sync.dma_start(out[qh * 64:(qh + 1) * 64, :], rss[qh][:])

        def v_blocks(j0, j1, vps):
            for j in range(j0, j1):
                vp = vps.tile([128, HPC * DH], F32, name="vp", tag="vp")
                for i in range(DC):
                    nc.tensor.matmul(
                        vp[:], xts[i][:, j * 128:(j + 1) * 128], wv_sb[i][:],
                        start=(i == 0), stop=(i == DC - 1))
                vt_view = vt_sb[j].rearrange("p (h e) -> p h e", h=HPC)
                nc.vector.tensor_copy(
                    vt_view[:, :, 0:DH], vp.rearrange("p (h e) -> p h e", h=HPC))
                nc.gpsimd.memset(vt_view[:, :, DH:DH + 1], 1.0)

        # phase A: q/k pair 0 + V projection, left S-half first and V-left
        # interleaved so the PE has work while the right-half x AllGather
        # is still on the wire
        with tc.tile_pool(name="qk0ps", bufs=1, space="PSUM") as qk0ps, \
             tc.tile_pool(name="vps", bufs=2, space="PSUM") as vps:
            proj_qk_pass(0, qk0ps, (0, 1))
            v_blocks(0, NKB // 2, vps)
            proj_qk_pass(0, qk0ps, (2, 3))
            v_blocks(NKB // 2, NKB, vps)

        # phase B onwards: attention pools (6 banks)
        with tc.tile_pool(name="att", bufs=4) as att, \
             tc.tile_pool(name="attsm", bufs=4) as attsm, \
             tc.tile_pool(name="scps", bufs=2, space="PSUM") as scps, \
             tc.tile_pool(name="ctxps", bufs=2, space="PSUM") as ctxps:

            # pair-0 attention
            for qt_i in range(NQT):
                for h in range(2):
                    attention(0, h, qt_i, scps, ctxps, att, attsm)

            # q/k pair 1 hides under pair-0 attention's ACT/DVE span
            with tc.tile_pool(name="qk1ps", bufs=2, space="PSUM") as qk1ps:
                proj_qk_seq(1, qk1ps)

            # pair-1 attention; out-projection + its ReduceScatter issued
            # immediately per finished q tile so the RS chunks spread out
            with tc.tile_pool(name="ph3ps", bufs=2, space="PSUM") as ph3ps, \
                 tc.tile_pool(name="ph3sb", bufs=3) as ph3sb:
                for qt_i in range(NQT):
                    for h in range(2):
                        attention(1, h, qt_i, scps, ctxps, att, attsm)
                    outproj(qt_i, ph3ps, ph3sb)

    nc.compile()
    return nc


_NC = None
_RUNNER = None
PROFILE = False
TRACE_CORES = (0,)
LAST_RESULT = None
LAST_TIMING = None


def _get_nc():
    global _NC
    if _NC is None:
        _NC = _build()
    return _NC


def _make_runner(nc):
    """Cached PJRT dispatcher: like bass2jax.run_bass_via_pjrt but the jit is
    traced/compiled once and reused, and the donated output buffers are
    created on device by a helper jit instead of being uploaded as host
    zeros (saves n_outs worth of host->device transfer per call)."""
    import time

    import jax
    import jax.numpy as jnp
    from jax.experimental.shard_map import shard_map
    from jax.sharding import Mesh, NamedSharding, PartitionSpec

    from concourse import bass2jax

    bass2jax.install_neuronx_cc_hook()

    partition_name = (
        nc.partition_id_tensor.name if nc.partition_id_tensor else None
    )
    in_names, out_names, out_avals, zero_specs = [], [], [], []
    for alloc in nc.m.functions[0].allocations:
        if not isinstance(alloc, mybir.MemoryLocationSet):
            continue
        name = alloc.memorylocations[0].name
        if alloc.kind == "ExternalInput":
            if name != partition_name:
                in_names.append(name)
        elif alloc.kind == "ExternalOutput":
            out_names.append(name)
            shape = tuple(alloc.tensor_shape)
            dtype = mybir.dt.np(alloc.dtype)
            out_avals.append(jax.core.ShapedArray(shape, dtype))
            zero_specs.append((shape, dtype))
    n_params = len(in_names)
    n_outs = len(out_names)
    all_in_names = in_names + out_names
    if partition_name is not None:
        all_in_names.append(partition_name)

    def _body(*args):
        operands = list(args)
        if partition_name is not None:
            operands.append(bass2jax.partition_id_tensor())
        outs = bass2jax._bass_exec_p.bind(
            *operands,
            out_avals=tuple(out_avals),
            in_names=tuple(all_in_names),
            out_names=tuple(out_names),
            lowering_input_output_aliases=(),
            sim_require_finite=True,
            sim_require_nnan=True,
            nc=nc,
        )
        return tuple(outs)

    devices = jax.devices()[:NCORES]
    mesh = Mesh(np.asarray(devices), ("core",))
    in_specs = (PartitionSpec("core"),) * (n_params + n_outs)
    out_specs = (PartitionSpec("core"),) * n_outs
    donate = tuple(range(n_params, n_params + n_outs))
    sharded = jax.jit(
        shard_map(_body, mesh=mesh, in_specs=in_specs, out_specs=out_specs,
                  check_rep=False),
        donate_argnums=donate, keep_unused=True)

    zsh = tuple(NamedSharding(mesh, PartitionSpec("core")) for _ in range(n_outs))
    make_zeros = jax.jit(
        lambda: tuple(jnp.zeros((NCORES * s[0], *s[1:]), d)
                      for s, d in zero_specs),
        out_shardings=zsh)

    def run(in_maps):
        global LAST_TIMING
        t0 = time.time()
        concat_in = [
            np.concatenate([np.asarray(m[nm]) for m in in_maps], axis=0)
            for nm in in_names]
        t1 = time.time()
        zeros = make_zeros()
        jax.block_until_ready(zeros)
        t2 = time.time()
        out_arrs = sharded(*concat_in, *zeros)
        out_np = [np.asarray(a) for a in out_arrs]
        t3 = time.time()
        LAST_TIMING = {"prep": t1 - t0, "zeros": t2 - t1, "exec": t3 - t2}
        return [
            {nm: out_np[i].reshape(NCORES, *out_avals[i].shape)[c]
             for i, nm in enumerate(out_names)}
            for c in range(NCORES)]

    return run


def kernel(x, Wq, Wk, Wv, Wo, bo):
    x = np.asarray(x, dtype=np.float32)
    Wq = np.asarray(Wq, dtype=np.float32)
    Wk = np.asarray(Wk, dtype=np.float32)
    Wv = np.asarray(Wv, dtype=np.float32)
    Wo = np.asarray(Wo, dtype=np.float32)
    bo = np.asarray(bo, dtype=np.float32)

    nc = _get_nc()

    in_maps = _prepare_in_maps(x, Wq, Wk, Wv, Wo, bo)

    global LAST_RESULT, _RUNNER
    if PROFILE:
        kw = dict(trace=True, trace_cores=list(TRACE_CORES))
        res = run_bass_kernel_spmd(
            nc, in_maps, core_ids=list(range(NCORES)), **kw)
        LAST_RESULT = res
        results = res.results
    else:
        if _RUNNER is None:
            _RUNNER = _make_runner(nc)
        results = _RUNNER(in_maps)

    out = np.zeros((B, S, D), np.float32)
    for c in range(NCORES):
        b, g = divmod(c, 4)
        o = results[c]["out"].astype(np.float32)
        for qh in range(2 * NQT):
            out[b, qh * 256 + 64 * g:qh * 256 + 64 * (g + 1)] = \
                o[qh * 64:(qh + 1) * 64]
    return out


def _prepare_in_maps(x, Wq, Wk, Wv, Wo, bo):
    import ml_dtypes
    bf16 = ml_dtypes.bfloat16

    xTs = [np.ascontiguousarray(x[b].T).astype(bf16) for b in range(B)]
    bo_row = np.ascontiguousarray(bo[None, :]).astype(bf16)
    zeros_row = np.zeros((1, D), bf16)

    in_maps = []
    for c in range(NCORES):
        b, g = divmod(c, 4)
        cs = slice(g * HPC * DH, (g + 1) * HPC * DH)
        if b == 0:
            blob = np.concatenate([Wq[:, cs], Wk[:, cs]], axis=0).astype(bf16)
        else:
            blob = np.concatenate(
                [Wv[:, cs], Wo[cs, :].reshape(D, HPC * DH)], axis=0).astype(bf16)
        in_maps.append({
            "x_u": np.ascontiguousarray(xTs[b][256 * g:256 * (g + 1), :]),
            "w_u": np.ascontiguousarray(blob),
            "bo_r": bo_row if g == 0 else zeros_row,
        })
    return in_maps


# revision 42
# speedup vs baseline: 1.0648x; 1.0648x over previous
"""Multi-head causal attention (B=2, S=2048, D=1024, H=16) on 8 trn2 cores.

Sharding: core c -> batch b=c//4, head-group g=c%4 (heads 4g..4g+3).

Host I/O is minimized: each core uploads only a distinct 1 MiB chunk of
xT[b] (AllGather over the 4-core batch group rebuilds the full xT on
device) and half of its head-group weight blob (pair AllGather over
{g, g+4} rebuilds the full blob). The out-projection partials are written
as fp16 and ReduceScatter(add)-ed over the batch group, so each core
downloads only its own 512-row slice of the final output.

Each core: Q/K/V projections for its heads from xT[b], causal attention in
transposed layout, row-parallel out-projection partial (bias is fed as
zeros to 3 of the 4 cores and applied on-device via a K=1 broadcast
matmul, so it survives the cross-core reduction exactly once).
"""

import numpy as np

import concourse.bass as bass
import concourse.tile as tile
import concourse.mybir as mybir
from concourse import bacc
from concourse.bass_utils import run_bass_kernel_spmd

B, S, D, H, DH = 2, 2048, 1024, 16, 64
NCORES = 8
HPC = 4          # heads per core
PAIRS = 2        # head pairs per core
QT = 512         # q tile (free dim of scoresT / PV matmuls)
KB = 128         # k block (partition dim of scoresT)
NQT = S // QT    # 4
NKB = S // KB    # 16
DC = D // 128    # 8 contraction chunks for projections
SCALE = 1.0 / np.sqrt(DH)

F32 = mybir.dt.float32
F16 = mybir.dt.float16
BF = mybir.dt.bfloat16

G4 = [[0, 1, 2, 3], [4, 5, 6, 7]]      # batch groups (reduce heads)
G2 = [[0, 4], [1, 5], [2, 6], [3, 7]]  # weight pairs (same head-group)

# bench-only ablation switches (set of strings)
ABLATE = set()


def _build():
    nc = bacc.Bacc("TRN2", target_bir_lowering=False, debug=False, num_devices=NCORES)

    # per-core distinct uploads (1 MiB each)
    x_u = nc.dram_tensor("x_u", [256, S], BF, kind="ExternalInput").ap()
    w_u = nc.dram_tensor("w_u", [2048, 256], BF, kind="ExternalInput").ap()
    bo_r = nc.dram_tensor("bo_r", [1, D], BF, kind="ExternalInput").ap()
    # per-core distinct download: rows 512g..512(g+1) of batch b, fp16
    out = nc.dram_tensor("out", [S // 4, D], F16, kind="ExternalOutput").ap()

    with tile.TileContext(nc) as tc, \
         tc.tile_pool(name="dram", bufs=1, space="DRAM") as dram, \
         tc.tile_pool(name="persist", bufs=1) as persist:
        # ---- internal DRAM (collective operands must not be I/O tensors) ----
        # x AllGather is split into S-halves so projections can start on the
        # left half while the right half is still on the wire.
        x_bl = dram.tile([256, S // 2], BF, name="x_bl")
        x_br = dram.tile([256, S // 2], BF, name="x_br")
        xT_l = dram.tile([D, S // 2], BF, name="xT_l")
        xT_r = dram.tile([D, S // 2], BF, name="xT_r")
        # weight pair exchange: b=0 cores upload [Wq; Wk] slices, b=1 cores
        # upload [Wv; Wo-flat]; one pair AllGather rebuilds the full blob
        w_b = dram.tile([2048, 256], BF, name="w_b")
        wfull = dram.tile([4096, 256], BF, name="wfull")
        # out-proj partials + ReduceScatter results, one pair per q tile so
        # each RS only depends on its own tile's writes
        parts = [dram.tile([QT, D], F16, name=f"part{t}") for t in range(NQT)]
        rss = [dram.tile([KB, D], F16, name=f"rs{t}") for t in range(NQT)]

        # bounces go on the scalar HWDGE ring so queued input loads (which
        # wait on the AllGather semaphores) can't head-of-line block them
        nc.scalar.dma_start(w_b[:], w_u[:])
        nc.scalar.dma_start(x_bl[:], x_u[:, 0:S // 2])
        nc.scalar.dma_start(x_br[:], x_u[:, S // 2:S])
        nc.gpsimd.collective_compute(
            "AllGather", mybir.AluOpType.bypass, replica_groups=G2,
            ins=[w_b.opt()], outs=[wfull.opt()])
        nc.gpsimd.collective_compute(
            "AllGather", mybir.AluOpType.bypass, replica_groups=G4,
            ins=[x_bl.opt()], outs=[xT_l.opt()])
        nc.gpsimd.collective_compute(
            "AllGather", mybir.AluOpType.bypass, replica_groups=G4,
            ins=[x_br.opt()], outs=[xT_r.opt()])

        # ---- persistent tiles ----
        qt_sb = [persist.tile([128, S], BF, name=f"qt{p}", tag=f"qt{p}") for p in range(PAIRS)]
        kt_sb = [persist.tile([128, S], BF, name=f"kt{p}", tag=f"kt{p}") for p in range(PAIRS)]
        # V' tiles: per s-block j, [128, 4*65]; head hl at cols 65*hl, ones col at 65*hl+64
        vt_sb = [persist.tile([128, HPC * (DH + 1)], BF, name=f"vt{j}", tag=f"vt{j}") for j in range(NKB)]
        ctx_sb = [persist.tile([128, S], BF, name=f"ctx{p}", tag=f"ctx{p}") for p in range(PAIRS)]
        wo_sb = [persist.tile([128, D], BF, name=f"wo{p}", tag=f"wo{p}") for p in range(PAIRS)]
        tri_sb = persist.tile([KB, KB], BF, name="tri", tag="tri")
        bo_sb = persist.tile([1, D], BF, name="bo", tag="bo")
        ones_sb = persist.tile([1, 128], BF, name="ones", tag="ones")

        xts = [persist.tile([128, S], BF, name=f"xts{i}", tag=f"xts{i}") for i in range(DC)]
        wq_sb = [persist.tile([128, HPC * DH], BF, name=f"wq{i}", tag=f"wq{i}") for i in range(DC)]
        wk_sb = [persist.tile([128, HPC * DH], BF, name=f"wk{i}", tag=f"wk{i}") for i in range(DC)]
        wv_sb = [persist.tile([128, HPC * DH], BF, name=f"wv{i}", tag=f"wv{i}") for i in range(DC)]

        # lower-triangular [128,128] mask built on device: keep where q-k >= 0
        tri_ones = persist.tile([KB, KB], BF, name="tri1", tag="tri1")
        nc.gpsimd.memset(tri_ones[:], 1.0)
        nc.gpsimd.affine_select(
            tri_sb[:], tri_ones[:], pattern=[[1, KB]],
            compare_op=mybir.AluOpType.is_ge, fill=0.0, base=0,
            channel_multiplier=-1)
        nc.sync.dma_start(bo_sb[:], bo_r[:])
        nc.gpsimd.memset(ones_sb[:], 1.0)
        for i in range(DC):
            nc.sync.dma_start(xts[i][:, 0:S // 2], xT_l[i * 128:(i + 1) * 128, :])
            nc.sync.dma_start(wq_sb[i][:], wfull[i * 128:(i + 1) * 128, :])
            nc.sync.dma_start(wk_sb[i][:], wfull[1024 + i * 128:1024 + (i + 1) * 128, :])
            nc.sync.dma_start(wv_sb[i][:], wfull[2048 + i * 128:2048 + (i + 1) * 128, :])
        for i in range(DC):
            nc.sync.dma_start(xts[i][:, S // 2:S], xT_r[i * 128:(i + 1) * 128, :])
        for p in range(PAIRS):
            # Wo slice rows 128p..128(p+1) of [256, D], stored flat as
            # [512, 256] rows 3072+512p.. of the blob
            nc.sync.dma_start(
                wo_sb[p].rearrange("q (u f) -> q u f", u=4),
                wfull[3072 + 512 * p:3072 + 512 * (p + 1), :].rearrange(
                    "(q u) f -> q u f", u=4))

        def proj_qk_pass(p, pool, sts):
            """q/k projection for pair p over q tiles `sts`, D-chunk-outer so
            matmuls chase the xT DMAs chunk by chunk. Tags are shared between
            passes so both passes fit in 4 psum banks (WAR deps via Tile)."""
            qps = {st: pool.tile([128, QT], F32, name=f"qps{st}", tag=f"qk{st % 2}")
                   for st in sts}
            kps = {st: pool.tile([128, QT], F32, name=f"kps{st}", tag=f"qk{2 + st % 2}")
                   for st in sts}
            for i in range(DC):
                for st in sts:
                    nc.tensor.matmul(
                        qps[st][:], wq_sb[i][:, p * 128:(p + 1) * 128],
                        xts[i][:, st * QT:(st + 1) * QT],
                        start=(i == 0), stop=(i == DC - 1))
                for st in sts:
                    nc.tensor.matmul(
                        kps[st][:], wk_sb[i][:, p * 128:(p + 1) * 128],
                        xts[i][:, st * QT:(st + 1) * QT],
                        start=(i == 0), stop=(i == DC - 1))
            for st in sts:
                nc.scalar.copy(qt_sb[p][:, st * QT:(st + 1) * QT], qps[st][:])
                nc.vector.tensor_copy(kt_sb[p][:, st * QT:(st + 1) * QT], kps[st][:])

        def proj_qk_seq(p, pool):
            """q/k projection, sequential psum (2 banks) — for overlap with
            attention of the other pair."""
            for st in range(NQT):
                qp = pool.tile([128, QT], F32, name="qp", tag="qkseq")
                for i in range(DC):
                    nc.tensor.matmul(
                        qp[:], wq_sb[i][:, p * 128:(p + 1) * 128],
                        xts[i][:, st * QT:(st + 1) * QT],
                        start=(i == 0), stop=(i == DC - 1))
                nc.scalar.copy(qt_sb[p][:, st * QT:(st + 1) * QT], qp[:])
                kp = pool.tile([128, QT], F32, name="kp", tag="qkseq")
                for i in range(DC):
                    nc.tensor.matmul(
                        kp[:], wk_sb[i][:, p * 128:(p + 1) * 128],
                        xts[i][:, st * QT:(st + 1) * QT],
                        start=(i == 0), stop=(i == DC - 1))
                nc.vector.tensor_copy(kt_sb[p][:, st * QT:(st + 1) * QT], kp[:])

        def attention(p, h, qt_i, scps, ctxps, att, attsm):
            hl = 2 * p + h
            r0, r1 = h * 64, h * 64 + 64
            q0 = qt_i * QT
            nkb = 4 * (qt_i + 1)
            cps = ctxps.tile([DH + 1, QT], F32, name="cps", tag="cps")
            for g0 in range(0, nkb, 2):
                sp = scps.tile([128, 2 * QT], F32, name="sp", tag="sp")
                for u in range(2):
                    kb = g0 + u
                    nc.tensor.matmul(
                        sp[:, u * QT:(u + 1) * QT],
                        kt_sb[p][r0:r1, kb * KB:(kb + 1) * KB],
                        qt_sb[p][r0:r1, q0:q0 + QT],
                        start=True, stop=True)
                pt = att.tile([128, 2 * QT], BF, name="pt", tag="pt")
                nc.scalar.activation(
                    pt[:], sp[:], mybir.ActivationFunctionType.Exp,
                    scale=float(SCALE))
                # causal masking: straddle groups are exactly g0==4qt (d=0,1)
                # and g0==4qt+2 (d=2,3): memset the dead rectangles (gpsimd),
                # multiply the [128,128] diagonal triangles (DVE)
                if "masks" in ABLATE:
                    pass
                elif g0 == 4 * qt_i:          # blocks d=0, d=1
                    nc.gpsimd.memset(pt[:, QT:QT + KB], 0.0)
                    for off in (0, QT + KB):
                        nc.vector.tensor_mul(
                            pt[:, off:off + KB], pt[:, off:off + KB], tri_sb[:])
                elif g0 == 4 * qt_i + 2:    # blocks d=2, d=3
                    nc.gpsimd.memset(pt[:, 0:2 * KB], 0.0)
                    nc.gpsimd.memset(pt[:, QT:QT + 3 * KB], 0.0)
                    for off in (2 * KB, QT + 3 * KB):
                        nc.vector.tensor_mul(
                            pt[:, off:off + KB], pt[:, off:off + KB], tri_sb[:])
                for u in range(2):
                    kb = g0 + u
                    nc.tensor.matmul(
                        cps[:],
                        vt_sb[kb][:, hl * (DH + 1):(hl + 1) * (DH + 1)],
                        pt[:, u * QT:(u + 1) * QT],
                        start=(kb == 0), stop=(kb == nkb - 1))
            # normalize: r = 1/l broadcast over the 64 ctx rows
            r_sb = attsm.tile([1, QT], F32, name="r_sb", tag="r")
            nc.vector.reciprocal(r_sb[:], cps[DH:DH + 1, :])
            rb = attsm.tile([64, QT], F32, name="rb", tag="rb")
            nc.gpsimd.partition_broadcast(rb[:], r_sb[:])
            nc.vector.tensor_mul(
                ctx_sb[p][r0:r1, q0:q0 + QT], cps[0:DH, :], rb[:])

        def outproj(qt_i, ph3ps, ph3sb):
            """partial out-projection rows for one q tile, bias folded in as a
            K=1 matmul; copy PSUM->SBUF (fp16 cast) split across ACT/DVE, then
            DMA to the partials buffer for the cross-core ReduceScatter."""
            for qb in range(qt_i * 4, qt_i * 4 + 4):
                os_ = ph3sb.tile([128, D], F16, name="os", tag="os")
                for nh in range(2):
                    op = ph3ps.tile([128, 512], F32, name="op", tag="op")
                    nc.tensor.matmul(
                        op[:], ones_sb[:], bo_sb[:, nh * 512:(nh + 1) * 512],
                        start=True, stop=False)
                    for p in range(PAIRS):
                        nc.tensor.matmul(
                            op[:], ctx_sb[p][:, qb * 128:(qb + 1) * 128],
                            wo_sb[p][:, nh * 512:(nh + 1) * 512],
                            start=False, stop=(p == PAIRS - 1))
                    dst = os_[:, nh * 512:(nh + 1) * 512]
                    if nh == 0:
                        nc.scalar.copy(dst, op[:])
                    else:
                        nc.vector.tensor_copy(dst, op[:])
                lr = (qb - qt_i * 4) * 128
                nc.sync.dma_start(parts[qt_i][lr:lr + 128, :], os_[:])

        def rs_tile(qt_i):
            """reduce this q tile's partials across the batch group; rank g
            keeps rows 128g..128(g+1), written to output rows 128*qt_i.."""
            nc.gpsimd.collective_compute(
                "ReduceScatter", mybir.AluOpType.add, replica_groups=G4,
                ins=[parts[qt_i].opt()], outs=[rss[qt_i].opt()])
            nc.sync.dma_start(out[qt_i * KB:(qt_i + 1) * KB, :], rss[qt_i][:])

        def v_blocks(j0, j1, vps):
            for j in range(j0, j1):
                vp = vps.tile([128, HPC * DH], F32, name="vp", tag="vp")
                for i in range(DC):
                    nc.tensor.matmul(
                        vp[:], xts[i][:, j * 128:(j + 1) * 128], wv_sb[i][:],
                        start=(i == 0), stop=(i == DC - 1))
                vt_view = vt_sb[j].rearrange("p (h e) -> p h e", h=HPC)
                nc.vector.tensor_copy(
                    vt_view[:, :, 0:DH], vp.rearrange("p (h e) -> p h e", h=HPC))
                nc.gpsimd.memset(vt_view[:, :, DH:DH + 1], 1.0)

        # phase A: q/k pair 0 + V projection, left S-half first and V-left
        # interleaved so the PE has work while the right-half x AllGather
        # is still on the wire
        with tc.tile_pool(name="qk0ps", bufs=1, space="PSUM") as qk0ps, \
             tc.tile_pool(name="vps", bufs=2, space="PSUM") as vps:
            proj_qk_pass(0, qk0ps, (0, 1))
            v_blocks(0, NKB // 2, vps)
            proj_qk_pass(0, qk0ps, (2, 3))
            v_blocks(NKB // 2, NKB, vps)

        # phase B onwards: attention pools (6 banks)
        with tc.tile_pool(name="att", bufs=4) as att, \
             tc.tile_pool(name="attsm", bufs=4) as attsm, \
             tc.tile_pool(name="scps", bufs=2, space="PSUM") as scps, \
             tc.tile_pool(name="ctxps", bufs=2, space="PSUM") as ctxps:

            # pair-0 attention
            for qt_i in range(NQT):
                for h in range(2):
                    attention(0, h, qt_i, scps, ctxps, att, attsm)

            # q/k pair 1 hides under pair-0 attention's ACT/DVE span
            with tc.tile_pool(name="qk1ps", bufs=2, space="PSUM") as qk1ps:
                proj_qk_seq(1, qk1ps)

            # pair-1 attention; out-projection + its ReduceScatter issued
            # immediately per finished q tile so the RS chunks spread out
            with tc.tile_pool(name="ph3ps", bufs=2, space="PSUM") as ph3ps, \
                 tc.tile_pool(name="ph3sb", bufs=3) as ph3sb:
                for qt_i in range(NQT):
                    for h in range(2):
                        attention(1, h, qt_i, scps, ctxps, att, attsm)
                    outproj(qt_i, ph3ps, ph3sb)
                    rs_tile(qt_i)

    nc.compile()
    return nc


_NC = None
_RUNNER = None
PROFILE = False
TRACE_CORES = (0,)
LAST_RESULT = None
LAST_TIMING = None


def _get_nc():
    global _NC
    if _NC is None:
        _NC = _build()
    return _NC


def _make_runner(nc):
    """Cached PJRT dispatcher: like bass2jax.run_bass_via_pjrt but the jit is
    traced/compiled once and reused, and the donated output buffers are
    created on device by a helper jit instead of being uploaded as host
    zeros (saves n_outs worth of host->device transfer per call)."""
    import time

    import jax
    import jax.numpy as jnp
    from jax.experimental.shard_map import shard_map
    from jax.sharding import Mesh, NamedSharding, PartitionSpec

    from concourse import bass2jax

    bass2jax.install_neuronx_cc_hook()

    partition_name = (
        nc.partition_id_tensor.name if nc.partition_id_tensor else None
    )
    in_names, out_names, out_avals, zero_specs = [], [], [], []
    for alloc in nc.m.functions[0].allocations:
        if not isinstance(alloc, mybir.MemoryLocationSet):
            continue
        name = alloc.memorylocations[0].name
        if alloc.kind == "ExternalInput":
            if name != partition_name:
                in_names.append(name)
        elif alloc.kind == "ExternalOutput":
            out_names.append(name)
            shape = tuple(alloc.tensor_shape)
            dtype = mybir.dt.np(alloc.dtype)
            out_avals.append(jax.core.ShapedArray(shape, dtype))
            zero_specs.append((shape, dtype))
    n_params = len(in_names)
    n_outs = len(out_names)
    all_in_names = in_names + out_names
    if partition_name is not None:
        all_in_names.append(partition_name)

    def _body(*args):
        operands = list(args)
        if partition_name is not None:
            operands.append(bass2jax.partition_id_tensor())
        outs = bass2jax._bass_exec_p.bind(
            *operands,
            out_avals=tuple(out_avals),
            in_names=tuple(all_in_names),
            out_names=tuple(out_names),
            lowering_input_output_aliases=(),
            sim_require_finite=True,
            sim_require_nnan=True,
            nc=nc,
        )
        return tuple(outs)

    devices = jax.devices()[:NCORES]
    mesh = Mesh(np.asarray(devices), ("core",))
    in_specs = (PartitionSpec("core"),) * (n_params + n_outs)
    out_specs = (PartitionSpec("core"),) * n_outs
    donate = tuple(range(n_params, n_params + n_outs))
    sharded = jax.jit(
        shard_map(_body, mesh=mesh, in_specs=in_specs, out_specs=out_specs,
                  check_rep=False),
        donate_argnums=donate, keep_unused=True)

    zsh = tuple(NamedSharding(mesh, PartitionSpec("core")) for _ in range(n_outs))
    make_zeros = jax.jit(
        lambda: tuple(jnp.zeros((NCORES * s[0], *s[1:]), d)
                      for s, d in zero_specs),
        out_shardings=zsh)

    def run(in_maps):
        global LAST_TIMING
        t0 = time.time()
        concat_in = [
            np.concatenate([np.asarray(m[nm]) for m in in_maps], axis=0)
            for nm in in_names]
        t1 = time.time()
        zeros = make_zeros()
        jax.block_until_ready(zeros)
        t2 = time.time()
        out_arrs = sharded(*concat_in, *zeros)
        out_np = [np.asarray(a) for a in out_arrs]
        t3 = time.time()
        LAST_TIMING = {"prep": t1 - t0, "zeros": t2 - t1, "exec": t3 - t2}
        return [
            {nm: out_np[i].reshape(NCORES, *out_avals[i].shape)[c]
             for i, nm in enumerate(out_names)}
            for c in range(NCORES)]

    return run


def kernel(x, Wq, Wk, Wv, Wo, bo):
    x = np.asarray(x, dtype=np.float32)
    Wq = np.asarray(Wq, dtype=np.float32)
    Wk = np.asarray(Wk, dtype=np.float32)
    Wv = np.asarray(Wv, dtype=np.float32)
    Wo = np.asarray(Wo, dtype=np.float32)
    bo = np.asarray(bo, dtype=np.float32)

    nc = _get_nc()

    in_maps = _prepare_in_maps(x, Wq, Wk, Wv, Wo, bo)

    global LAST_RESULT, _RUNNER
    if PROFILE:
        kw = dict(trace=True, trace_cores=list(TRACE_CORES))
        res = run_bass_kernel_spmd(
            nc, in_maps, core_ids=list(range(NCORES)), **kw)
        LAST_RESULT = res
        results = res.results
    else:
        if _RUNNER is None:
            _RUNNER = _make_runner(nc)
        results = _RUNNER(in_maps)

    out = np.zeros((B, S, D), np.float32)
    for c in range(NCORES):
        b, g = divmod(c, 4)
        o = results[c]["out"].astype(np.float32)
        for qt in range(NQT):
            out[b, qt * 512 + 128 * g:qt * 512 + 128 * (g + 1)] = \
                o[qt * 128:(qt + 1) * 128]
    return out


def _prepare_in_maps(x, Wq, Wk, Wv, Wo, bo):
    import ml_dtypes
    bf16 = ml_dtypes.bfloat16

    xTs = [np.ascontiguousarray(x[b].T).astype(bf16) for b in range(B)]
    bo_row = np.ascontiguousarray(bo[None, :]).astype(bf16)
    zeros_row = np.zeros((1, D), bf16)

    in_maps = []
    for c in range(NCORES):
        b, g = divmod(c, 4)
        cs = slice(g * HPC * DH, (g + 1) * HPC * DH)
        if b == 0:
            blob = np.concatenate([Wq[:, cs], Wk[:, cs]], axis=0).astype(bf16)
        else:
            blob = np.concatenate(
                [Wv[:, cs], Wo[cs, :].reshape(D, HPC * DH)], axis=0).astype(bf16)
        in_maps.append({
            "x_u": np.ascontiguousarray(xTs[b][256 * g:256 * (g + 1), :]),
            "w_u": np.ascontiguousarray(blob),
            "bo_r": bo_row if g == 0 else zeros_row,
        })
    return in_maps
